# revision 25
# baseline (speedup 1.0000x reference)
"""Multi-head attention (B=4, S=2048, E=1024, H=16, causal) on 8 Trainium2 cores.

Sharding: core = (batch b, head-group g) — 4 batches x 2 groups of 8 heads.
Each core computes q/k/v projections for its batch restricted to its 8 heads,
causal attention for those heads, and a partial output projection over its
512 ctx columns.  The host sums the two partials per batch and adds all
output-side bias terms analytically (softmax rows sum to one, so the v-bias
passes through attention unchanged: out += o_b + v_b @ o_w.T).

On-device layouts (per core):
  qT/kT: [head_dim 512 -> 4 tiles of 128, token 2048]  (2 heads per tile)
  v_aug: [token -> 16 tiles of 128, 8 heads x (64 dims + ones col)]
  scores are computed transposed (k^T q per head, contraction dim 64,
  two heads row-tiled concurrently in the PE array), softmax is max-free
  (scores are O(+-8), exp cannot overflow fp32), causal masking is
  structural: fully-masked tiles are skipped, diagonal tiles get a
  memset + affine_select staircase fill.
  attn @ v is computed as v_aug^T @ expT giving ctx^T plus the softmax
  row-sum in one matmul (ones column of v_aug).

The program is software-pipelined over token column blocks n=0..3:
projections for block n feed the attention column block j=n and the
output projection for the same block, so the PE-heavy projection work of
block n+1 overlaps the ScalarE-heavy exp work of block n.  Score tiles
are paired into [128,1024] PSUM tensors so one exp instruction covers
two k-tiles (halves ScalarE per-op overhead).
"""

import os
import sys

for _p in ("/opt/trn_rl_repo", "/root/.axon_site/_ro/trn_rl_repo"):
    if os.path.isdir(_p) and _p not in sys.path:
        sys.path.append(_p)

import numpy as np
import ml_dtypes

import concourse.bacc as bacc
import concourse.mybir as mybir
from concourse import tile
from concourse import bass_utils
from concourse.bass import ts

BF16 = ml_dtypes.bfloat16
F32 = mybir.dt.float32
BF = mybir.dt.bfloat16
AFT = mybir.ActivationFunctionType
ALU = mybir.AluOpType

B, S, E = 4, 2048, 1024
H, D = 16, 64
G = 512            # head dims per core (8 heads)
KC = E // 128      # contraction chunks for projections
NM = G // 128      # m-tiles of the group dim
NJ = S // 512      # 512-wide token column blocks
NT = S // 128      # 128-wide token tiles

_NC = None


def _build():
    nc = bacc.Bacc("TRN2", target_bir_lowering=False, debug=False, num_devices=8)

    xq = nc.dram_tensor("xq", (E, S), BF, kind="ExternalInput").ap()
    xk = nc.dram_tensor("xk", (E, S), BF, kind="ExternalInput").ap()
    xv = nc.dram_tensor("xv", (E, S), BF, kind="ExternalInput").ap()
    wq = nc.dram_tensor("wq", (E, G), BF, kind="ExternalInput").ap()
    wk = nc.dram_tensor("wk", (E, G), BF, kind="ExternalInput").ap()
    wv = nc.dram_tensor("wv", (E, G), BF, kind="ExternalInput").ap()
    wo = nc.dram_tensor("wo", (G, E), BF, kind="ExternalInput").ap()
    qb = nc.dram_tensor("qb", (128, NM), F32, kind="ExternalInput").ap()
    kb = nc.dram_tensor("kb", (128, NM), F32, kind="ExternalInput").ap()
    sel = nc.dram_tensor("sel", (4, G), BF, kind="ExternalInput").ap()
    fT = nc.dram_tensor("fT", (E, S), F32, kind="ExternalOutput").ap()

    with tile.TileContext(nc) as tc:
        with (
            tc.tile_pool(name="cst", bufs=2) as cst,
            tc.tile_pool(name="wsb", bufs=24) as wsb,
            tc.tile_pool(name="xs", bufs=6) as xsp,
            tc.tile_pool(name="qt", bufs=8) as qtp,
            tc.tile_pool(name="va", bufs=16) as vap,
            tc.tile_pool(name="ctx", bufs=4) as ctxp,
            tc.tile_pool(name="exp", bufs=10) as expp,
            tc.tile_pool(name="wo", bufs=4) as wop,
            tc.tile_pool(name="fin", bufs=4) as finp,
            tc.tile_pool(name="rb", bufs=4) as rbp,
            tc.tile_pool(name="tmp", bufs=3) as tmpp,
            tc.tile_pool(name="ps", bufs=3, space="PSUM") as psp,
            tc.tile_pool(name="psc", bufs=2, space="PSUM") as pscp,
        ):
            qb_t = cst.tile([128, NM], F32, tag="cst")
            kb_t = cst.tile([128, NM], F32, tag="cst")
            sel_sb = cst.tile([68, G], BF, tag="sel", name="sel_sb")

            zero_fill = nc.gpsimd.to_reg(0.0)

            # Warm the ScalarE Exp table at kernel start: the table load that
            # precedes the first Exp does not reliably complete before the
            # first exp executes on a cold core (first-execution races showed
            # garbage softmax weights in the earliest attention column), so
            # trigger it long before the real exps.
            warm = cst.tile([1, 8], F32, tag="warm", name="warm")
            nc.vector.memset(warm[:, :], 0.0)
            nc.scalar.activation(warm[:, :], warm[:, :], AFT.Exp)
            # causal staircase mask: tri[:, 512-128(r+1):512] masks a
            # diagonal 128-block preceded by 128r fully-masked columns
            tri = cst.tile([128, 512], BF, tag="tri", name="tri")
            nc.vector.memset(tri[:, 0:384], 0.0)
            nc.vector.memset(tri[:, 384:512], 1.0)
            nc.gpsimd.affine_select(
                out=tri[:, 384:512], in_=tri[:, 384:512],
                pattern=[[1, 128]], compare_op=ALU.is_ge,
                fill=zero_fill, base=0, channel_multiplier=-1)
            # constant ones-slots pattern for v_aug cols [64..72) per head
            ones_c = cst.tile([128, 8 * 68], BF, tag="ones", name="ones_c")
            ones_c3 = ones_c[:, :].rearrange("p (h x) -> p h x", h=8)
            nc.vector.memset(ones_c3[:, :, 64:68], 0.0)
            for h in range(8):
                nc.vector.memset(
                    ones_c3[:, h : h + 1, 64 + (h % 4) : 65 + (h % 4)], 1.0)

            qT = [qtp.tile([128, S], BF, tag="qt", name=f"qT{m}") for m in range(NM)]
            kT = [qtp.tile([128, S], BF, tag="qt", name=f"kT{m}") for m in range(NM)]
            ctxT = [ctxp.tile([128, S], BF, tag="ctx", name=f"ctxT{m}")
                    for m in range(NM)]
            v_aug = [None] * NT

            nc.gpsimd.dma_start(qb_t[:, :], qb[:, :])
            nc.gpsimd.dma_start(kb_t[:, :], kb[:, :])
            nc.gpsimd.dma_start(sel_sb[64:68, :], sel[:, :])

            # weights stay resident for the whole kernel
            wq_sb = [wsb.tile([128, G], BF, tag="w", name=f"wq{kc}") for kc in range(KC)]
            wk_sb = [wsb.tile([128, G], BF, tag="w", name=f"wk{kc}") for kc in range(KC)]
            wv_sb = [wsb.tile([128, G], BF, tag="w", name=f"wv{kc}") for kc in range(KC)]
            for kc in range(KC):
                nc.gpsimd.dma_start(wq_sb[kc][:, :], wq[ts(kc, 128), :])
                nc.gpsimd.dma_start(wk_sb[kc][:, :], wk[ts(kc, 128), :])
                nc.gpsimd.dma_start(wv_sb[kc][:, :], wv[ts(kc, 128), :])
            wo_sb = [wop.tile([128, E], BF, tag="wo", name=f"wo{ec}") for ec in range(NM)]
            for ec in range(NM):
                nc.gpsimd.dma_start(wo_sb[ec][:, :], wo[ts(ec, 128), :])

            def proj_qk(n, x_ap, w_sb, dst, bias_t, scale, dma_eng):
                xsb = xsp.tile([128, KC * 512], BF, tag="xs", name="xsb")
                xs3 = xsb[:, :].rearrange("p (k c) -> p k c", k=KC)
                if n == 0:
                    # fine-grained loads, alternating HWDGE queues, so the
                    # first matmul chains are not DMA-trigger-paced
                    for kc in range(KC):
                        eng = dma_eng if kc % 2 == 0 else (
                            nc.scalar if dma_eng is nc.sync else nc.sync)
                        eng.dma_start(
                            xs3[:, kc, :], x_ap[ts(kc, 128), ts(n, 512)])
                else:
                    dma_eng.dma_start(
                        xs3[:, :, :],
                        x_ap[:, ts(n, 512)].rearrange("(k p) c -> p k c", p=128))
                xs = [xs3[:, kc, :] for kc in range(KC)]
                for mp in range(2):
                    psd = psp.tile([128, 1024], F32, tag="ps1024", name="psd")
                    ps0, ps1 = psd[:, 0:512], psd[:, 512:1024]
                    for kc in range(KC):
                        nc.tensor.matmul(
                            ps0, w_sb[kc][:, ts(2 * mp, 128)], xs[kc],
                            start=(kc == 0), stop=(kc == KC - 1))
                        nc.tensor.matmul(
                            ps1, w_sb[kc][:, ts(2 * mp + 1, 128)], xs[kc],
                            start=(kc == 0), stop=(kc == KC - 1))
                    for mh, ps in ((0, ps0), (1, ps1)):
                        m = 2 * mp + mh
                        nc.vector.tensor_scalar(
                            dst[m][:, ts(n, 512)], ps,
                            scale, bias_t[:, m : m + 1],
                            ALU.mult, ALU.add)

            def proj_v(tg):
                xsb = xsp.tile([128, KC * 512], BF, tag="xs", name="xsvb")
                xs3 = xsb[:, :].rearrange("p (k c) -> p k c", k=KC)
                if tg == 0:
                    for kc in range(KC):
                        nc.sync.dma_start(
                            xs3[:, kc, :], xv[ts(kc, 128), ts(tg, 512)])
                else:
                    nc.sync.dma_start(
                        xs3[:, :, :],
                        xv[:, ts(tg, 512)].rearrange("(k p) c -> p k c", p=128))
                xs = [xs3[:, kc, :] for kc in range(KC)]
                for tp in range(2):
                    psd = psp.tile([128, 1024], F32, tag="ps1024", name="psd")
                    ps0, ps1 = psd[:, 0:512], psd[:, 512:1024]
                    for kc in range(KC):
                        nc.tensor.matmul(
                            ps0, xs[kc][:, ts(2 * tp, 128)], wv_sb[kc][:, :],
                            start=(kc == 0), stop=(kc == KC - 1))
                        nc.tensor.matmul(
                            ps1, xs[kc][:, ts(2 * tp + 1, 128)], wv_sb[kc][:, :],
                            start=(kc == 0), stop=(kc == KC - 1))
                    for th, ps in ((0, ps0), (1, ps1)):
                        tt = 4 * tg + 2 * tp + th
                        va = vap.tile([128, 8 * 68], BF, tag="va", name=f"va{tt}")
                        va3 = va[:, :].rearrange("p (h x) -> p h x", h=8)
                        ps3 = ps.rearrange("p (h x) -> p h x", h=8)
                        nc.vector.tensor_copy(va3[:, :, 0:64], ps3[:, :, :])
                        nc.vector.tensor_copy(
                            va3[:, :, 64:68], ones_c3[:, :, 64:68])
                        v_aug[tt] = va

            def attention_block(hp, j, rs_j):
                ni = 4 * j + 4          # causal: tk tiles 0..4j+3 (always even)
                np2 = ni // 2
                cA = pscp.tile([68, 512], F32, tag="psc", name="cA")
                cB = pscp.tile([68, 512], F32, tag="psc", name="cB")
                hA, hB = 2 * hp, 2 * hp + 1
                for ip in range(np2):
                    i0, i1 = 2 * ip, 2 * ip + 1
                    sA = psp.tile([128, 1024], F32, tag="ps1024", name="sA")
                    sB = psp.tile([128, 1024], F32, tag="ps1024", name="sB")
                    for half, i in ((0, i0), (1, i1)):
                        nc.tensor.matmul(
                            sA[:, ts(half, 512)],
                            kT[hp][0:64, ts(i, 128)], qT[hp][0:64, ts(j, 512)],
                            start=True, stop=True)
                        nc.tensor.matmul(
                            sB[:, ts(half, 512)],
                            kT[hp][64:128, ts(i, 128)], qT[hp][64:128, ts(j, 512)],
                            start=True, stop=True, tile_position=(64, 0))
                    eA = expp.tile([128, 1024], BF, tag="exp", name="eA")
                    eB = expp.tile([128, 1024], BF, tag="exp", name="eB")
                    nc.scalar.activation(eA[:, :], sA[:, :], AFT.Exp)
                    nc.scalar.activation(eB[:, :], sB[:, :], AFT.Exp)
                    for half, i in ((0, i0), (1, i1)):
                        r = i - 4 * j
                        if r < 0:
                            continue
                        for e in (eA, eB):
                            # zero everything left of / above the diagonal:
                            # multiply by the staircase-mask slice
                            w = 128 * (r + 1)
                            nc.vector.tensor_mul(
                                e[:, 512 * half : 512 * half + w],
                                e[:, 512 * half : 512 * half + w],
                                tri[:, 512 - w : 512])
                    for half, i in ((0, i0), (1, i1)):
                        nc.tensor.matmul(
                            cA[:, :], v_aug[i][:, hA * 68 : hA * 68 + 68],
                            eA[:, ts(half, 512)],
                            start=(i == 0), stop=(i == ni - 1))
                        nc.tensor.matmul(
                            cB[:, :], v_aug[i][:, hB * 68 : hB * 68 + 68],
                            eB[:, ts(half, 512)],
                            start=(i == 0), stop=(i == ni - 1))
                # evacuate unnormalized ctx to SBUF; accumulate row-sums
                # (each head occupies its own partition in the 64..72 band)
                for c, half in ((cA, 0), (cB, 1)):
                    nc.vector.tensor_add(
                        rs_j[64:68, :], rs_j[64:68, :], c[64:68, :])
                    if half == 0:
                        nc.vector.tensor_copy(ctxT[hp][0:64, ts(j, 512)], c[0:64, :])
                    else:
                        tm = tmpp.tile([64, 512], BF, tag="tmp", name="tm")
                        nc.vector.tensor_copy(tm[:, :], c[0:64, :])
                        nc.gpsimd.dma_start(ctxT[hp][64:128, ts(j, 512)], tm[:, :])

            def oproj_block(q4, ec_order=(0, 1, 2, 3)):
                for jtp in range(4):
                    psd = psp.tile([128, 1024], F32, tag="ps1024", name="psd")
                    ps0, ps1 = psd[:, 0:512], psd[:, 512:1024]
                    for ei, ec in enumerate(ec_order):
                        nc.tensor.matmul(
                            ps0, wo_sb[ec][:, ts(2 * jtp, 128)],
                            ctxT[ec][:, ts(q4, 512)],
                            start=(ei == 0), stop=(ei == NM - 1))
                        nc.tensor.matmul(
                            ps1, wo_sb[ec][:, ts(2 * jtp + 1, 128)],
                            ctxT[ec][:, ts(q4, 512)],
                            start=(ei == 0), stop=(ei == NM - 1))
                    st = finp.tile([128, 1024], F32, tag="fin", name="st")
                    nc.scalar.activation(st[:, :], psd[:, :], AFT.Copy)
                    nc.gpsimd.dma_start(
                        fT[256 * jtp : 256 * jtp + 256, ts(q4, 512)].rearrange(
                            "(a p) c -> p a c", p=128),
                        st[:, :].rearrange("p (a c) -> p a c", a=2))

            def normalize_half(j, rs_x, hpp):
                # heads of hp = 2*hpp, 2*hpp+1
                rec = rbp.tile([68, 512], F32, tag="rec", name="rec", bufs=3)
                nc.vector.reciprocal(rec[64:68, :], rs_x[64:68, :])
                recb = rbp.tile([68, 512], BF, tag="recb", name="recb", bufs=3)
                nc.vector.tensor_copy(recb[64:68, :], rec[64:68, :])
                psn = psp.tile([128, 1024], F32, tag="ps1024", name="psn")
                for hh in range(2):
                    hp = 2 * hpp + hh
                    nc.tensor.matmul(
                        psn[:, ts(hh, 512)],
                        sel_sb[64:68, ts(hp, 128)], recb[64:68, :],
                        start=True, stop=True, tile_position=(64, 0))
                    nc.vector.tensor_mul(
                        ctxT[hp][:, ts(j, 512)], ctxT[hp][:, ts(j, 512)],
                        psn[:, ts(hh, 512)])

            # ---- software pipeline over token column blocks ----------------
            def proj_block(n):
                proj_qk(n, xq, wq_sb, qT, qb_t, 0.125, nc.sync)
                proj_qk(n, xk, wk_sb, kT, kb_t, 1.0, nc.scalar)
                proj_v(n)

            proj_block(0)
            for n in range(NJ):
                rs_a = rbp.tile([68, 512], F32, tag="rs", name=f"rsa{n}", bufs=4)
                rs_b = rbp.tile([68, 512], F32, tag="rs", name=f"rsb{n}", bufs=4)
                nc.vector.memset(rs_a[64:68, :], 0.0)
                nc.vector.memset(rs_b[64:68, :], 0.0)
                if n + 1 < NJ:
                    attention_block(0, n, rs_a)
                    attention_block(1, n, rs_a)
                    attention_block(2, n, rs_b)
                    attention_block(3, n, rs_b)
                    proj_block(n + 1)
                    normalize_half(n, rs_a, 0)
                    normalize_half(n, rs_b, 1)
                    oproj_block(n)
                else:
                    attention_block(0, n, rs_a)
                    attention_block(1, n, rs_a)
                    attention_block(2, n, rs_b)
                    attention_block(3, n, rs_b)
                    normalize_half(n, rs_a, 0)
                    normalize_half(n, rs_b, 1)
                    oproj_block(n)

    nc.compile()
    return nc


def _get_nc():
    global _NC
    if _NC is None:
        _NC = _build()
    return _NC


def build_in_maps(inputs):
    query = np.asarray(inputs["query"], np.float32)
    key = np.asarray(inputs["key"], np.float32)
    value = np.asarray(inputs["value"], np.float32)
    q_w = np.asarray(inputs["q_w"], np.float32)
    q_b = np.asarray(inputs["q_b"], np.float32)
    k_w = np.asarray(inputs["k_w"], np.float32)
    k_b = np.asarray(inputs["k_b"], np.float32)
    v_w = np.asarray(inputs["v_w"], np.float32)
    o_w = np.asarray(inputs["o_w"], np.float32)

    xqT = [np.ascontiguousarray(query[b].T).astype(BF16) for b in range(B)]
    xkT = [np.ascontiguousarray(key[b].T).astype(BF16) for b in range(B)]
    xvT = [np.ascontiguousarray(value[b].T).astype(BF16) for b in range(B)]

    wqT, wkT, wvT, woT, qbt, kbt = [], [], [], [], [], []
    for g in range(2):
        gs = slice(g * G, (g + 1) * G)
        wqT.append(np.ascontiguousarray(q_w[gs, :].T).astype(BF16))
        wkT.append(np.ascontiguousarray(k_w[gs, :].T).astype(BF16))
        wvT.append(np.ascontiguousarray(v_w[gs, :].T).astype(BF16))
        woT.append(np.ascontiguousarray(o_w[:, gs].T).astype(BF16))
        qbt.append(
            np.ascontiguousarray((q_b[gs] / 8.0).reshape(NM, 128).T).astype(
                np.float32
            )
        )
        kbt.append(
            np.ascontiguousarray(k_b[gs].reshape(NM, 128).T).astype(np.float32)
        )

    sel_np = np.zeros((4, G), np.float32)
    for k in range(4):
        for p in range(G):
            hp, pp = p // 128, p % 128
            if k == (2 * hp + (pp // 64)) % 4:
                sel_np[k, p] = 1.0
    sel_np = sel_np.astype(BF16)

    in_maps = []
    for b in range(B):
        for g in range(2):
            in_maps.append(
                {
                    "xq": xqT[b],
                    "xk": xkT[b],
                    "xv": xvT[b],
                    "wq": wqT[g],
                    "wk": wkT[g],
                    "wv": wvT[g],
                    "wo": woT[g],
                    "qb": qbt[g],
                    "kb": kbt[g],
                    "sel": sel_np,
                }
            )

    return in_maps


def kernel(**inputs):
    nc = _get_nc()
    in_maps = build_in_maps(inputs)
    res = bass_utils.run_bass_kernel_spmd(nc, in_maps, core_ids=list(range(8)))

    o_b = np.asarray(inputs["o_b"], np.float32)
    v_b = np.asarray(inputs["v_b"], np.float32)
    o_w = np.asarray(inputs["o_w"], np.float32)
    corr = (o_b + v_b @ o_w.T).astype(np.float32)  # softmax rows sum to 1
    out = np.empty((B, S, E), np.float32)
    for b in range(B):
        acc = res.results[2 * b]["fT"] + res.results[2 * b + 1]["fT"]
        out[b] = acc.T + corr[None, :]
    return out


# revision 26
# speedup vs baseline: 1.0075x; 1.0075x over previous
"""Multi-head attention (B=4, S=2048, E=1024, H=16, causal) on 8 Trainium2 cores.

Sharding: core = (batch b, head-group g) — 4 batches x 2 groups of 8 heads.
Each core computes q/k/v projections for its batch restricted to its 8 heads,
causal attention for those heads, and a partial output projection over its
512 ctx columns.  The host sums the two partials per batch and adds all
output-side bias terms analytically (softmax rows sum to one, so the v-bias
passes through attention unchanged: out += o_b + v_b @ o_w.T).

On-device layouts (per core):
  qT/kT: [head_dim 512 -> 4 tiles of 128, token 2048]  (2 heads per tile)
  v_aug: [token -> 16 tiles of 128, 8 heads x (64 dims + ones col)]
  scores are computed transposed (k^T q per head, contraction dim 64,
  two heads row-tiled concurrently in the PE array), softmax is max-free
  (scores are O(+-8), exp cannot overflow fp32), causal masking is
  structural: fully-masked tiles are skipped, diagonal tiles get a
  memset + affine_select staircase fill.
  attn @ v is computed as v_aug^T @ expT giving ctx^T plus the softmax
  row-sum in one matmul (ones column of v_aug).

The program is software-pipelined over token column blocks n=0..3:
projections for block n feed the attention column block j=n and the
output projection for the same block, so the PE-heavy projection work of
block n+1 overlaps the ScalarE-heavy exp work of block n.  Score tiles
are paired into [128,1024] PSUM tensors so one exp instruction covers
two k-tiles (halves ScalarE per-op overhead).
"""

import os
import sys

for _p in ("/opt/trn_rl_repo", "/root/.axon_site/_ro/trn_rl_repo"):
    if os.path.isdir(_p) and _p not in sys.path:
        sys.path.append(_p)

import numpy as np
import ml_dtypes

import concourse.bacc as bacc
import concourse.mybir as mybir
from concourse import tile
from concourse import bass_utils
from concourse.bass import ts

BF16 = ml_dtypes.bfloat16
F32 = mybir.dt.float32
BF = mybir.dt.bfloat16
AFT = mybir.ActivationFunctionType
ALU = mybir.AluOpType

B, S, E = 4, 2048, 1024
H, D = 16, 64
G = 512            # head dims per core (8 heads)
KC = E // 128      # contraction chunks for projections
NM = G // 128      # m-tiles of the group dim
NJ = S // 512      # 512-wide token column blocks
NT = S // 128      # 128-wide token tiles

_NC = None


def _build():
    nc = bacc.Bacc("TRN2", target_bir_lowering=False, debug=False, num_devices=8)

    xq = nc.dram_tensor("xq", (E, S), BF, kind="ExternalInput").ap()
    xk = nc.dram_tensor("xk", (E, S), BF, kind="ExternalInput").ap()
    xv = nc.dram_tensor("xv", (E, S), BF, kind="ExternalInput").ap()
    wq = nc.dram_tensor("wq", (E, G), BF, kind="ExternalInput").ap()
    wk = nc.dram_tensor("wk", (E, G), BF, kind="ExternalInput").ap()
    wv = nc.dram_tensor("wv", (E, G), BF, kind="ExternalInput").ap()
    wo = nc.dram_tensor("wo", (G, E), BF, kind="ExternalInput").ap()
    qb = nc.dram_tensor("qb", (128, NM), F32, kind="ExternalInput").ap()
    kb = nc.dram_tensor("kb", (128, NM), F32, kind="ExternalInput").ap()
    sel = nc.dram_tensor("sel", (4, G), BF, kind="ExternalInput").ap()
    fT = nc.dram_tensor("fT", (E, S), F32, kind="ExternalOutput").ap()

    with tile.TileContext(nc) as tc:
        with (
            tc.tile_pool(name="cst", bufs=2) as cst,
            tc.tile_pool(name="wsb", bufs=24) as wsb,
            tc.tile_pool(name="xs", bufs=6) as xsp,
            tc.tile_pool(name="qt", bufs=8) as qtp,
            tc.tile_pool(name="va", bufs=16) as vap,
            tc.tile_pool(name="ctx", bufs=4) as ctxp,
            tc.tile_pool(name="exp", bufs=10) as expp,
            tc.tile_pool(name="wo", bufs=4) as wop,
            tc.tile_pool(name="fin", bufs=4) as finp,
            tc.tile_pool(name="rb", bufs=4) as rbp,
            tc.tile_pool(name="tmp", bufs=4) as tmpp,
            tc.tile_pool(name="ps", bufs=3, space="PSUM") as psp,
            tc.tile_pool(name="psc", bufs=2, space="PSUM") as pscp,
        ):
            qb_t = cst.tile([128, NM], F32, tag="cst")
            kb_t = cst.tile([128, NM], F32, tag="cst")
            sel_sb = cst.tile([68, G], BF, tag="sel", name="sel_sb")

            zero_fill = nc.gpsimd.to_reg(0.0)

            # Warm the ScalarE Exp table at kernel start: the table load that
            # precedes the first Exp does not reliably complete before the
            # first exp executes on a cold core (first-execution races showed
            # garbage softmax weights in the earliest attention column), so
            # trigger it long before the real exps.
            warm = cst.tile([1, 8], F32, tag="warm", name="warm")
            nc.vector.memset(warm[:, :], 0.0)
            nc.scalar.activation(warm[:, :], warm[:, :], AFT.Exp)
            # constant ones-slots pattern for v_aug cols [64..72) per head
            ones_c = cst.tile([128, 8 * 68], BF, tag="ones", name="ones_c")
            ones_c3 = ones_c[:, :].rearrange("p (h x) -> p h x", h=8)
            nc.vector.memset(ones_c3[:, :, 64:68], 0.0)
            for h in range(8):
                nc.vector.memset(
                    ones_c3[:, h : h + 1, 64 + (h % 4) : 65 + (h % 4)], 1.0)

            qT = [qtp.tile([128, S], BF, tag="qt", name=f"qT{m}") for m in range(NM)]
            kT = [qtp.tile([128, S], BF, tag="qt", name=f"kT{m}") for m in range(NM)]
            ctxT = [ctxp.tile([128, S], BF, tag="ctx", name=f"ctxT{m}")
                    for m in range(NM)]
            v_aug = [None] * NT

            nc.gpsimd.dma_start(qb_t[:, :], qb[:, :])
            nc.gpsimd.dma_start(kb_t[:, :], kb[:, :])
            nc.gpsimd.dma_start(sel_sb[64:68, :], sel[:, :])

            # weights stay resident for the whole kernel
            wq_sb = [wsb.tile([128, G], BF, tag="w", name=f"wq{kc}") for kc in range(KC)]
            wk_sb = [wsb.tile([128, G], BF, tag="w", name=f"wk{kc}") for kc in range(KC)]
            wv_sb = [wsb.tile([128, G], BF, tag="w", name=f"wv{kc}") for kc in range(KC)]
            for kc in range(KC):
                nc.gpsimd.dma_start(wq_sb[kc][:, :], wq[ts(kc, 128), :])
                nc.gpsimd.dma_start(wk_sb[kc][:, :], wk[ts(kc, 128), :])
                nc.gpsimd.dma_start(wv_sb[kc][:, :], wv[ts(kc, 128), :])
            wo_sb = [wop.tile([128, E], BF, tag="wo", name=f"wo{ec}") for ec in range(NM)]
            for ec in range(NM):
                nc.gpsimd.dma_start(wo_sb[ec][:, :], wo[ts(ec, 128), :])

            def proj_qk(n, x_ap, w_sb, dst, bias_t, scale, dma_eng):
                xsb = xsp.tile([128, KC * 512], BF, tag="xs", name="xsb")
                xs3 = xsb[:, :].rearrange("p (k c) -> p k c", k=KC)
                if n == 0:
                    # fine-grained loads, alternating HWDGE queues, so the
                    # first matmul chains are not DMA-trigger-paced
                    for kc in range(KC):
                        eng = dma_eng if kc % 2 == 0 else (
                            nc.scalar if dma_eng is nc.sync else nc.sync)
                        eng.dma_start(
                            xs3[:, kc, :], x_ap[ts(kc, 128), ts(n, 512)])
                else:
                    dma_eng.dma_start(
                        xs3[:, :, :],
                        x_ap[:, ts(n, 512)].rearrange("(k p) c -> p k c", p=128))
                xs = [xs3[:, kc, :] for kc in range(KC)]
                for mp in range(2):
                    psd = psp.tile([128, 1024], F32, tag="ps1024", name="psd")
                    ps0, ps1 = psd[:, 0:512], psd[:, 512:1024]
                    for kc in range(KC):
                        nc.tensor.matmul(
                            ps0, w_sb[kc][:, ts(2 * mp, 128)], xs[kc],
                            start=(kc == 0), stop=(kc == KC - 1))
                        nc.tensor.matmul(
                            ps1, w_sb[kc][:, ts(2 * mp + 1, 128)], xs[kc],
                            start=(kc == 0), stop=(kc == KC - 1))
                    for mh, ps in ((0, ps0), (1, ps1)):
                        m = 2 * mp + mh
                        nc.vector.tensor_scalar(
                            dst[m][:, ts(n, 512)], ps,
                            scale, bias_t[:, m : m + 1],
                            ALU.mult, ALU.add)

            def proj_v(tg):
                xsb = xsp.tile([128, KC * 512], BF, tag="xs", name="xsvb")
                xs3 = xsb[:, :].rearrange("p (k c) -> p k c", k=KC)
                if tg == 0:
                    for kc in range(KC):
                        nc.sync.dma_start(
                            xs3[:, kc, :], xv[ts(kc, 128), ts(tg, 512)])
                else:
                    nc.sync.dma_start(
                        xs3[:, :, :],
                        xv[:, ts(tg, 512)].rearrange("(k p) c -> p k c", p=128))
                xs = [xs3[:, kc, :] for kc in range(KC)]
                for tp in range(2):
                    psd = psp.tile([128, 1024], F32, tag="ps1024", name="psd")
                    ps0, ps1 = psd[:, 0:512], psd[:, 512:1024]
                    for kc in range(KC):
                        nc.tensor.matmul(
                            ps0, xs[kc][:, ts(2 * tp, 128)], wv_sb[kc][:, :],
                            start=(kc == 0), stop=(kc == KC - 1))
                        nc.tensor.matmul(
                            ps1, xs[kc][:, ts(2 * tp + 1, 128)], wv_sb[kc][:, :],
                            start=(kc == 0), stop=(kc == KC - 1))
                    for th, ps in ((0, ps0), (1, ps1)):
                        tt = 4 * tg + 2 * tp + th
                        va = vap.tile([128, 8 * 68], BF, tag="va", name=f"va{tt}")
                        va3 = va[:, :].rearrange("p (h x) -> p h x", h=8)
                        ps3 = ps.rearrange("p (h x) -> p h x", h=8)
                        nc.vector.tensor_copy(va3[:, :, 0:64], ps3[:, :, :])
                        nc.vector.tensor_copy(
                            va3[:, :, 64:68], ones_c3[:, :, 64:68])
                        v_aug[tt] = va

            def attention_block(hp, j, rs_j):
                ni = 4 * j + 4          # causal: tk tiles 0..4j+3 (always even)
                np2 = ni // 2
                cA = pscp.tile([68, 512], F32, tag="psc", name="cA")
                cB = pscp.tile([68, 512], F32, tag="psc", name="cB")
                hA, hB = 2 * hp, 2 * hp + 1
                for ip in range(np2):
                    i0, i1 = 2 * ip, 2 * ip + 1
                    sA = psp.tile([128, 1024], F32, tag="ps1024", name="sA")
                    sB = psp.tile([128, 1024], F32, tag="ps1024", name="sB")
                    for half, i in ((0, i0), (1, i1)):
                        nc.tensor.matmul(
                            sA[:, ts(half, 512)],
                            kT[hp][0:64, ts(i, 128)], qT[hp][0:64, ts(j, 512)],
                            start=True, stop=True)
                        nc.tensor.matmul(
                            sB[:, ts(half, 512)],
                            kT[hp][64:128, ts(i, 128)], qT[hp][64:128, ts(j, 512)],
                            start=True, stop=True, tile_position=(64, 0))
                    eA = expp.tile([128, 1024], BF, tag="exp", name="eA")
                    eB = expp.tile([128, 1024], BF, tag="exp", name="eB")
                    nc.scalar.activation(eA[:, :], sA[:, :], AFT.Exp)
                    nc.scalar.activation(eB[:, :], sB[:, :], AFT.Exp)
                    for half, i in ((0, i0), (1, i1)):
                        r = i - 4 * j
                        if r < 0:
                            continue
                        for e in (eA, eB):
                            # zero everything left of / above the diagonal in
                            # one pass: keep iff col - 128r - row >= 0
                            nc.gpsimd.affine_select(
                                out=e[:, 512 * half : 512 * half + 128 * (r + 1)],
                                in_=e[:, 512 * half : 512 * half + 128 * (r + 1)],
                                pattern=[[1, 128 * (r + 1)]],
                                compare_op=ALU.is_ge,
                                fill=zero_fill,
                                base=-128 * r,
                                channel_multiplier=-1)
                    for half, i in ((0, i0), (1, i1)):
                        nc.tensor.matmul(
                            cA[:, :], v_aug[i][:, hA * 68 : hA * 68 + 68],
                            eA[:, ts(half, 512)],
                            start=(i == 0), stop=(i == ni - 1))
                        nc.tensor.matmul(
                            cB[:, :], v_aug[i][:, hB * 68 : hB * 68 + 68],
                            eB[:, ts(half, 512)],
                            start=(i == 0), stop=(i == ni - 1))
                # evacuate unnormalized ctx to SBUF; accumulate row-sums
                # (each head occupies its own partition in the 64..72 band)
                for c, half in ((cA, 0), (cB, 1)):
                    nc.vector.tensor_add(
                        rs_j[64:68, :], rs_j[64:68, :], c[64:68, :])
                    if half == 0:
                        nc.vector.tensor_copy(ctxT[hp][0:64, ts(j, 512)], c[0:64, :])
                    else:
                        tm = tmpp.tile([64, 512], BF, tag="tmp", name="tm")
                        nc.vector.tensor_copy(tm[:, :], c[0:64, :])
                        nc.gpsimd.dma_start(ctxT[hp][64:128, ts(j, 512)], tm[:, :])

            def oproj_block(q4, ec_order=(0, 1, 2, 3)):
                for jtp in range(4):
                    psd = psp.tile([128, 1024], F32, tag="ps1024", name="psd")
                    ps0, ps1 = psd[:, 0:512], psd[:, 512:1024]
                    for ei, ec in enumerate(ec_order):
                        nc.tensor.matmul(
                            ps0, wo_sb[ec][:, ts(2 * jtp, 128)],
                            ctxT[ec][:, ts(q4, 512)],
                            start=(ei == 0), stop=(ei == NM - 1))
                        nc.tensor.matmul(
                            ps1, wo_sb[ec][:, ts(2 * jtp + 1, 128)],
                            ctxT[ec][:, ts(q4, 512)],
                            start=(ei == 0), stop=(ei == NM - 1))
                    st = finp.tile([128, 1024], F32, tag="fin", name="st")
                    nc.scalar.activation(st[:, :], psd[:, :], AFT.Copy)
                    nc.gpsimd.dma_start(
                        fT[256 * jtp : 256 * jtp + 256, ts(q4, 512)].rearrange(
                            "(a p) c -> p a c", p=128),
                        st[:, :].rearrange("p (a c) -> p a c", a=2))

            def normalize_half(j, rs_x, hpp):
                # heads of hp = 2*hpp, 2*hpp+1
                rec = rbp.tile([68, 512], F32, tag="rec", name="rec", bufs=3)
                nc.vector.reciprocal(rec[64:68, :], rs_x[64:68, :])
                recb = rbp.tile([68, 512], BF, tag="recb", name="recb", bufs=3)
                nc.vector.tensor_copy(recb[64:68, :], rec[64:68, :])
                psn = psp.tile([128, 1024], F32, tag="ps1024", name="psn")
                for hh in range(2):
                    hp = 2 * hpp + hh
                    nc.tensor.matmul(
                        psn[:, ts(hh, 512)],
                        sel_sb[64:68, ts(hp, 128)], recb[64:68, :],
                        start=True, stop=True, tile_position=(64, 0))
                    nc.vector.tensor_mul(
                        ctxT[hp][:, ts(j, 512)], ctxT[hp][:, ts(j, 512)],
                        psn[:, ts(hh, 512)])

            # ---- software pipeline over token column blocks ----------------
            def proj_block(n):
                proj_qk(n, xq, wq_sb, qT, qb_t, 0.125, nc.sync)
                proj_qk(n, xk, wk_sb, kT, kb_t, 1.0, nc.scalar)
                proj_v(n)

            proj_block(0)
            for n in range(NJ):
                rs_a = rbp.tile([68, 512], F32, tag="rs", name=f"rsa{n}", bufs=4)
                rs_b = rbp.tile([68, 512], F32, tag="rs", name=f"rsb{n}", bufs=4)
                nc.vector.memset(rs_a[64:68, :], 0.0)
                nc.vector.memset(rs_b[64:68, :], 0.0)
                if n + 1 < NJ:
                    attention_block(0, n, rs_a)
                    attention_block(1, n, rs_a)
                    attention_block(2, n, rs_b)
                    attention_block(3, n, rs_b)
                    proj_block(n + 1)
                    normalize_half(n, rs_a, 0)
                    normalize_half(n, rs_b, 1)
                    oproj_block(n)
                else:
                    attention_block(0, n, rs_a)
                    attention_block(1, n, rs_a)
                    attention_block(2, n, rs_b)
                    attention_block(3, n, rs_b)
                    normalize_half(n, rs_a, 0)
                    normalize_half(n, rs_b, 1)
                    oproj_block(n)

    nc.compile()
    return nc


def _get_nc():
    global _NC
    if _NC is None:
        _NC = _build()
    return _NC


def build_in_maps(inputs):
    query = np.asarray(inputs["query"], np.float32)
    key = np.asarray(inputs["key"], np.float32)
    value = np.asarray(inputs["value"], np.float32)
    q_w = np.asarray(inputs["q_w"], np.float32)
    q_b = np.asarray(inputs["q_b"], np.float32)
    k_w = np.asarray(inputs["k_w"], np.float32)
    k_b = np.asarray(inputs["k_b"], np.float32)
    v_w = np.asarray(inputs["v_w"], np.float32)
    o_w = np.asarray(inputs["o_w"], np.float32)

    xqT = [np.ascontiguousarray(query[b].T).astype(BF16) for b in range(B)]
    xkT = [np.ascontiguousarray(key[b].T).astype(BF16) for b in range(B)]
    xvT = [np.ascontiguousarray(value[b].T).astype(BF16) for b in range(B)]

    wqT, wkT, wvT, woT, qbt, kbt = [], [], [], [], [], []
    for g in range(2):
        gs = slice(g * G, (g + 1) * G)
        wqT.append(np.ascontiguousarray(q_w[gs, :].T).astype(BF16))
        wkT.append(np.ascontiguousarray(k_w[gs, :].T).astype(BF16))
        wvT.append(np.ascontiguousarray(v_w[gs, :].T).astype(BF16))
        woT.append(np.ascontiguousarray(o_w[:, gs].T).astype(BF16))
        qbt.append(
            np.ascontiguousarray((q_b[gs] / 8.0).reshape(NM, 128).T).astype(
                np.float32
            )
        )
        kbt.append(
            np.ascontiguousarray(k_b[gs].reshape(NM, 128).T).astype(np.float32)
        )

    sel_np = np.zeros((4, G), np.float32)
    for k in range(4):
        for p in range(G):
            hp, pp = p // 128, p % 128
            if k == (2 * hp + (pp // 64)) % 4:
                sel_np[k, p] = 1.0
    sel_np = sel_np.astype(BF16)

    in_maps = []
    for b in range(B):
        for g in range(2):
            in_maps.append(
                {
                    "xq": xqT[b],
                    "xk": xkT[b],
                    "xv": xvT[b],
                    "wq": wqT[g],
                    "wk": wkT[g],
                    "wv": wvT[g],
                    "wo": woT[g],
                    "qb": qbt[g],
                    "kb": kbt[g],
                    "sel": sel_np,
                }
            )

    return in_maps


def kernel(**inputs):
    nc = _get_nc()
    in_maps = build_in_maps(inputs)
    res = bass_utils.run_bass_kernel_spmd(nc, in_maps, core_ids=list(range(8)))

    o_b = np.asarray(inputs["o_b"], np.float32)
    v_b = np.asarray(inputs["v_b"], np.float32)
    o_w = np.asarray(inputs["o_w"], np.float32)
    corr = (o_b + v_b @ o_w.T).astype(np.float32)  # softmax rows sum to 1
    out = np.empty((B, S, E), np.float32)
    for b in range(B):
        acc = res.results[2 * b]["fT"] + res.results[2 * b + 1]["fT"]
        out[b] = acc.T + corr[None, :]
    return out
